# revision 8
# baseline (speedup 1.0000x reference)
"""Trainium2 Bass kernel for nn_AttentionBlock (biased dense attention).

Math:  x' = x + phi_degree + phi_3d_sum
       S  = (x' Wq)(x' Wk)^T * scaling + phi_spd + phi_edge + phi_3d
       out = softmax(S, axis=-1) @ (x' Wv)

Strategy (8 cores, sequence parallel on q):
  - Host prep: xp = x' (tiny O(n*d) add), A = scaling * Wq @ Wk.T, fp16
    hi/lo splits of xp/A/Wv, and per-core TRANSPOSED row-shards of the
    three phi bias matrices (phi[qshard, :].T -> [n, n_loc]) so the
    device streams them with contiguous 2KB rows directly in the [k, q]
    orientation.
  - Each core holds x'^T (via PE identity-matmul transposes) and
    V = x' Wv resident in SBUF, computes B^T = A^T x'_loc^T, then for
    its q-shard computes S^T[k, q] tiles = xt-block.T @ B^T-chunk + bias.
    Computing S TRANSPOSED means exp(S^T) tiles are directly usable as
    the stationary operand of the P@V matmul - no on-chip transposes of
    the attention matrix at all.
  - Matmuls contract fp16 hi/lo pairs with 3 cross terms (hi*hi + hi*lo
    + lo*hi) accumulated in fp32 PSUM: fp32-grade accuracy at the fp16
    matmul rate (4x the fp32 rate).
  - Bias = phi_spd + phi_edge + phi_3d is summed for free by chained
    accum-DMA (SDMA inline fp32 adds), added to S^T psum by one DVE op.
  - exp(S - 12) on ScalarE (the e^-12 cancels in softmax normalization,
    keeping P in fp16 range for pv_mode="f16").
  - softmax denominators come for free from a ones-column appended to V.

kernel(**inputs) -> full [8192, 256] fp32 output.
"""

import contextlib

import numpy as np

import concourse.bacc as bacc
import concourse.tile as tile
from concourse import mybir
from concourse.bass_utils import run_bass_kernel_spmd
from concourse.masks import make_identity

N_FULL = 8192
D = 256
CORES = 8
SCALING = 0.0625

f32 = mybir.dt.float32
f16 = mybir.dt.float16

PV_MODE = "f16"  # "f16" (fast, ~5e-4) or "f32" (accurate, ~4e-5)


def build_attention_nc(n, n_loc, d=D, cores=CORES, reps=1, pv_mode=PV_MODE):
    """Build the SPMD Bass program (one program, runs on all cores)."""
    assert n % 512 == 0 and n_loc % 128 == 0 and d == 256
    QCH = min(512, n_loc)  # q-chunk: free dim of S^T tiles
    n_qc = n_loc // QCH
    n_kb = n // 128
    n_db = d // 128  # 2
    KSLAB = 8 if pv_mode == "f16" else 4  # k-blocks per phi DMA slab
    ASLAB = 4
    assert n_kb % KSLAB == 0
    pvdt = f16 if pv_mode == "f16" else f32
    vw = d + 2  # ones col at [256], zero col at [257] (even free dim)

    nc = bacc.Bacc("TRN2", target_bir_lowering=False, debug=False, num_devices=cores)

    def param(name, shape, dt=f32):
        return nc.declare_dram_parameter(name, shape, dt, isOutput=False)

    xp_h = param("xp_hi", [n, d], f16)
    xp_l = param("xp_lo", [n, d], f16)
    xpl_h = param("xp_loc_hi", [n_loc, d], f16)
    xpl_l = param("xp_loc_lo", [n_loc, d], f16)
    a_h = param("a_hi", [d, d], f16)
    a_l = param("a_lo", [d, d], f16)
    wv_h = param("wv_hi", [d, d], f16)
    wv_l = param("wv_lo", [d, d], f16)
    phis = [
        param(nm, [n, n_loc]) for nm in ("phi_spd_t", "phi_edge_t", "phi_3d_t")
    ]
    out = nc.declare_dram_parameter("out", [n_loc, d], f32, isOutput=True)

    with tile.TileContext(nc) as tc:
        loop_ctx = tc.For_i(0, reps, 1) if reps > 1 else contextlib.nullcontext()
        with loop_ctx, tc.tile_pool(name="res", bufs=1) as res:
            ident = res.tile([128, 128], f16)
            make_identity(nc, ident)
            bias12 = res.tile([128, 1], f32)
            nc.vector.memset(bias12, -12.0)
            a_sb = [res.tile([128, n_db, d], f16, name=f"a_sb{i}") for i in range(2)]
            wv_sb = [res.tile([128, n_db, d], f16, name=f"wv_sb{i}") for i in range(2)]
            for t_, src in (
                (a_sb[0], a_h),
                (a_sb[1], a_l),
                (wv_sb[0], wv_h),
                (wv_sb[1], wv_l),
            ):
                nc.sync.dma_start(out=t_, in_=src.rearrange("(b p) j -> p b j", p=128))

            # resident fp16 hi/lo pairs: x'^T (full), x'_loc^T, B^T; V(+ones)
            xt = [
                [res.tile([128, n], f16, name=f"xt{b}_{i}") for i in range(2)]
                for b in range(n_db)
            ]
            xtl = [
                [res.tile([128, n_loc], f16, name=f"xtl{b}_{i}") for i in range(2)]
                for b in range(n_db)
            ]
            bt = [
                [res.tile([128, n_loc], f16, name=f"bt{b}_{i}") for i in range(2)]
                for b in range(n_db)
            ]
            v_sb = res.tile([128, n_kb, vw], pvdt)

            # ---- Phase A: load xp hi/lo, transpose to x'^T hi/lo ----------
            psA_ctx = tc.tile_pool(name="psA", bufs=2, space="PSUM")
            psA = psA_ctx.__enter__()

            def transpose_in(src_dram, nrows, dst, hl):
                aslab = min(ASLAB, nrows // 128)
                src = src_dram.rearrange("(s a p) d -> s p a d", p=128, a=aslab)
                with tc.tile_pool(name="pha", bufs=3) as pha:
                    for s in range(nrows // (128 * aslab)):
                        slab = pha.tile([128, aslab, d], f16, tag="xpslab")
                        nc.sync.dma_start(out=slab, in_=src[s])
                        for db in range(n_db):
                            pt_ = psA.tile([128, 128 * aslab], f32, tag="xtp")
                            for a in range(aslab):
                                nc.tensor.matmul(
                                    pt_[:, a * 128 : (a + 1) * 128],
                                    slab[:, a, db * 128 : (db + 1) * 128],
                                    ident,
                                    start=True,
                                    stop=True,
                                )
                            nc.scalar.copy(
                                out=dst[db][hl][
                                    :, s * 128 * aslab : (s + 1) * 128 * aslab
                                ],
                                in_=pt_,
                            )

            transpose_in(xp_h, n, xt, 0)
            transpose_in(xp_l, n, xt, 1)
            transpose_in(xpl_h, n_loc, xtl, 0)
            transpose_in(xpl_l, n_loc, xtl, 1)

            # 3-term fp16 hi/lo contraction: hi*hi + hi*lo + lo*hi
            TERMS = ((0, 0), (0, 1), (1, 0))

            def mm3(out_ps, lhs_pairs, rhs_pairs):
                nblk = len(lhs_pairs)
                for blk in range(nblk):
                    for ti, (li, ri) in enumerate(TERMS):
                        nc.tensor.matmul(
                            out_ps,
                            lhs_pairs[blk][li],
                            rhs_pairs[blk][ri],
                            start=(blk == 0 and ti == 0),
                            stop=(blk == nblk - 1 and ti == len(TERMS) - 1),
                        )

            # ---- Phase B: B^T = A^T x'_loc^T ; V = x' Wv ------------------
            for db2 in range(n_db):
                for q0 in range(0, n_loc, QCH):
                    pb = psA.tile([128, QCH], f32, tag="btp")
                    mm3(
                        pb,
                        [
                            (
                                a_sb[0][:, d1, db2 * 128 : (db2 + 1) * 128],
                                a_sb[1][:, d1, db2 * 128 : (db2 + 1) * 128],
                            )
                            for d1 in range(n_db)
                        ],
                        [
                            (
                                xtl[d1][0][:, q0 : q0 + QCH],
                                xtl[d1][1][:, q0 : q0 + QCH],
                            )
                            for d1 in range(n_db)
                        ],
                    )
                    nc.scalar.copy(out=bt[db2][0][:, q0 : q0 + QCH], in_=pb)
                    nc.vector.tensor_sub(
                        bt[db2][1][:, q0 : q0 + QCH],
                        pb,
                        bt[db2][0][:, q0 : q0 + QCH],
                    )

            for kb in range(n_kb):
                pv = psA.tile([128, d], f32, tag="vp")
                mm3(
                    pv,
                    [
                        (
                            xt[d1][0][:, kb * 128 : (kb + 1) * 128],
                            xt[d1][1][:, kb * 128 : (kb + 1) * 128],
                        )
                        for d1 in range(n_db)
                    ],
                    [(wv_sb[0][:, d1, :], wv_sb[1][:, d1, :]) for d1 in range(n_db)],
                )
                nc.scalar.copy(out=v_sb[:, kb, :d], in_=pv)
            nc.vector.memset(v_sb[:, :, d : d + 1], 1.0)
            nc.vector.memset(v_sb[:, :, d + 1 : d + 2], 0.0)
            psA_ctx.__exit__(None, None, None)

            # ---- Phase C: S^T tiles -> exp -> @V --------------------------
            phi_r = [
                p.rearrange("(sb b p) q -> sb p b q", p=128, b=KSLAB) for p in phis
            ]
            with (
                tc.tile_pool(name="phC", bufs=3) as phC,
                tc.tile_pool(name="sbC", bufs=3) as sbC,
                tc.tile_pool(name="psS", bufs=3, space="PSUM") as psS,
                tc.tile_pool(name="psO", bufs=1, space="PSUM") as psO,
            ):
                for qc in range(n_qc):
                    out_ps = [
                        psO.tile([128, vw], f32, tag=f"out{t}", name=f"outp{qc}_{t}")
                        for t in range(QCH // 128)
                    ]
                    phi_slab = None
                    for kb in range(n_kb):
                        if kb % KSLAB == 0:
                            sb_i = kb // KSLAB
                            phi_slab = phC.tile([128, KSLAB, QCH], f32, tag="phisum")
                            for i, pr in enumerate(phi_r):
                                nc.gpsimd.dma_start(
                                    out=phi_slab,
                                    in_=pr[sb_i][:, :, qc * QCH : (qc + 1) * QCH],
                                    accum_op=(
                                        mybir.AluOpType.bypass
                                        if i == 0
                                        else mybir.AluOpType.add
                                    ),
                                )
                        s_ps = psS.tile([128, QCH], f32, tag="s")
                        mm3(
                            s_ps,
                            [
                                (
                                    xt[db][0][:, kb * 128 : (kb + 1) * 128],
                                    xt[db][1][:, kb * 128 : (kb + 1) * 128],
                                )
                                for db in range(n_db)
                            ],
                            [
                                (
                                    bt[db][0][:, qc * QCH : (qc + 1) * QCH],
                                    bt[db][1][:, qc * QCH : (qc + 1) * QCH],
                                )
                                for db in range(n_db)
                            ],
                        )
                        nc.vector.tensor_add(s_ps, s_ps, phi_slab[:, kb % KSLAB, :])
                        pt = sbC.tile([128, QCH], pvdt, tag="pt")
                        nc.scalar.activation(
                            out=pt,
                            in_=s_ps,
                            func=mybir.ActivationFunctionType.Exp,
                            bias=bias12,
                            scale=1.0,
                        )
                        for t in range(QCH // 128):
                            nc.tensor.matmul(
                                out_ps[t],
                                pt[:, t * 128 : (t + 1) * 128],
                                v_sb[:, kb, :],
                                start=(kb == 0),
                                stop=(kb == n_kb - 1),
                            )
                    for t in range(QCH // 128):
                        rs = sbC.tile([128, 1], f32, tag="rs")
                        nc.vector.reciprocal(rs, out_ps[t][:, d : d + 1])
                        ob = sbC.tile([128, d], f32, tag="ob")
                        nc.vector.tensor_scalar_mul(ob, out_ps[t][:, :d], rs)
                        r0 = qc * QCH + t * 128
                        nc.sync.dma_start(out=out[r0 : r0 + 128, :], in_=ob)
    nc.compile()
    return nc


def _split16(a):
    hi = a.astype(np.float16)
    lo = (a - hi.astype(np.float32)).astype(np.float16)
    return hi, lo


def _make_in_maps(xp, A, Wv, phi_spd, phi_edge, phi_3d, n_loc, cores=CORES):
    xp_hi, xp_lo = _split16(xp)
    a_hi, a_lo = _split16(A)
    wv_hi, wv_lo = _split16(Wv)
    in_maps = []
    for c in range(cores):
        r0, r1 = c * n_loc, (c + 1) * n_loc
        in_maps.append(
            {
                "xp_hi": xp_hi,
                "xp_lo": xp_lo,
                "xp_loc_hi": np.ascontiguousarray(xp_hi[r0:r1]),
                "xp_loc_lo": np.ascontiguousarray(xp_lo[r0:r1]),
                "a_hi": a_hi,
                "a_lo": a_lo,
                "wv_hi": wv_hi,
                "wv_lo": wv_lo,
                "phi_spd_t": np.ascontiguousarray(phi_spd[r0:r1].T),
                "phi_edge_t": np.ascontiguousarray(phi_edge[r0:r1].T),
                "phi_3d_t": np.ascontiguousarray(phi_3d[r0:r1].T),
            }
        )
    return in_maps


_CACHED_NC = {}


def _get_nc(n, n_loc):
    key = (n, n_loc)
    if key not in _CACHED_NC:
        _CACHED_NC[key] = build_attention_nc(n, n_loc)
    return _CACHED_NC[key]


def kernel(x, phi_degree, phi_3d_sum, phi_3d, phi_spd, phi_edge, Wq, Wk, Wv):
    x = np.asarray(x, dtype=np.float32)
    phi_degree = np.asarray(phi_degree, dtype=np.float32)
    phi_3d_sum = np.asarray(phi_3d_sum, dtype=np.float32)
    phi_3d = np.asarray(phi_3d, dtype=np.float32)
    phi_spd = np.asarray(phi_spd, dtype=np.float32)
    phi_edge = np.asarray(phi_edge, dtype=np.float32)
    Wq = np.asarray(Wq, dtype=np.float32)
    Wk = np.asarray(Wk, dtype=np.float32)
    Wv = np.asarray(Wv, dtype=np.float32)

    n = x.shape[0]
    n_loc = n // CORES
    xp = x + phi_degree + phi_3d_sum
    A = (SCALING * (Wq.astype(np.float64) @ Wk.astype(np.float64).T)).astype(
        np.float32
    )

    nc = _get_nc(n, n_loc)
    in_maps = _make_in_maps(xp, A, Wv, phi_spd, phi_edge, phi_3d, n_loc)
    res = run_bass_kernel_spmd(nc, in_maps, list(range(CORES)))
    return np.concatenate([res.results[c]["out"] for c in range(CORES)], axis=0)
